# revision 66
# baseline (speedup 1.0000x reference)
"""GQA (grouped-query attention) Trainium2 kernel, 8-core SPMD.

Sharding: TP=4 over kv-heads x DP=2 over batch  (core = b*4 + g).
Each core computes, for its batch b and kv-head g (q-heads 4g..4g+3):
  QKV projections -> RoPE -> causal softmax(QK^T)V -> partial x@Wo
entirely in transposed layout (feature dim on SBUF partitions), then the
host sums the 4 partial Wo outputs per batch (the TP all-reduce).

Single fused pipeline over the 4 query/sequence blocks of 512:
  proj(sb) -> attention(qb=sb) -> Wo-chunk(qb-1) quarters interleaved
between attention heads so the PE never drains.

Dataflow notes:
 - all matmul operands in bf16 (full PE rate, halves HBM traffic)
 - causal structure: strictly-upper k-blocks skipped; the 4 diagonal
   k-blocks use narrowed moving operands (512/384/256/128 queries) and a
   multiplicative triangular mask after exp
 - softmax runs in S^T[k,q] orientation; denominators: exp tiles are
   accumulated in bf16 into two partial tiles (even k-blocks on the vector
   engine, odd on GPSIMD), reduced across partitions per (head, q-block)
   with gpsimd.partition_all_reduce — the PE does no softmax bookkeeping
 - no max-subtraction: scores are bounded (~+-5) for this problem size
 - V^T -> V[k,dv] reorientation via DMA transpose (16-bit), not the PE
"""

import math
import sys

import numpy as np

if "/opt/trn_rl_repo" not in sys.path:
    sys.path.insert(0, "/opt/trn_rl_repo")

import ml_dtypes

B, S, D = 2, 2048, 2048
HQ, HKV, DH = 16, 4, 128
G = HQ // HKV            # q-heads per kv-head = 4
NCORES = 8
ROPE_THETA = 10000.0
SCALE = 1.0 / math.sqrt(DH)

SB = 512                 # wide column block (moving operand)
NSB = S // SB            # 4
ND = D // 128            # 16 contraction tiles
NKB = S // 128           # 16 key blocks

_CACHE = {}


def _build_nc():
    import concourse.bass as bass
    import concourse.mybir as mybir
    import concourse.tile as tile
    from concourse import bacc

    f32 = mybir.dt.float32
    bf16 = mybir.dt.bfloat16
    AF = mybir.ActivationFunctionType

    nc = bacc.Bacc(
        trn_type="TRN2", target_bir_lowering=False, debug=False,
        num_devices=NCORES,
    )

    xt_d = nc.dram_tensor("xt", [D, S], bf16, kind="ExternalInput").ap()
    wqkv_d = nc.dram_tensor("wqkv", [D, (G + 2) * DH], bf16, kind="ExternalInput").ap()
    wot_d = nc.dram_tensor("wot", [G * DH, D], bf16, kind="ExternalInput").ap()
    cos_d = nc.dram_tensor("cost", [DH, S], bf16, kind="ExternalInput").ap()
    sin_d = nc.dram_tensor("sints", [DH, S], bf16, kind="ExternalInput").ap()
    msk_d = nc.dram_tensor("masks", [128, SB], bf16, kind="ExternalInput").ap()
    y_d = nc.dram_tensor("y", [S, D], bf16, kind="ExternalOutput").ap()

    from contextlib import ExitStack

    with tile.TileContext(nc) as tc, ExitStack() as stack, \
            nc.allow_low_precision(reason="bf16 matmul operands"):
        persist = stack.enter_context(tc.tile_pool(name="persist", bufs=1))

        # persistent SBUF tensors (per-block tiles: no reliance on sub-tile
        # dependency tracking)
        xts = [[persist.tile([128, SB], bf16, name=f"x{s}_{i}", tag=f"x{s}_{i}")
                for i in range(ND)] for s in range(NSB)]
        wqkv = [persist.tile([128, (G + 2) * DH], bf16, name=f"w{i}", tag=f"w{i}")
                for i in range(ND)]
        wot = [persist.tile([128, D], bf16, name=f"wo{h}", tag=f"wo{h}")
               for h in range(G)]
        qrt = [[persist.tile([128, SB], bf16, name=f"q{h}_{s}", tag=f"q{h}_{s}")
                for s in range(NSB)] for h in range(G)]
        krt = [persist.tile([128, SB], bf16, name=f"k{s}", tag=f"k{s}")
               for s in range(NSB)]
        vsb = [persist.tile([128, DH], bf16, name=f"v{k}", tag=f"v{k}")
               for k in range(NKB)]
        a_t = [[persist.tile([128, SB], bf16, name=f"a{h}_{s}", tag=f"a{h}_{s}")
                for s in range(NSB)] for h in range(G)]
        cost = [persist.tile([128, SB], bf16, name=f"cost{s}", tag=f"cost{s}")
                for s in range(NSB)]
        sint = [persist.tile([128, SB], bf16, name=f"sint{s}", tag=f"sint{s}")
                for s in range(NSB)]
        mask = persist.tile([128, SB], bf16, name="mask", tag="mask")

        # working rings (SBUF)
        p_pool = stack.enter_context(tc.tile_pool(name="pp", bufs=4))
        dacc_pool = stack.enter_context(tc.tile_pool(name="dap", bufs=2))
        vsbw_pool = stack.enter_context(tc.tile_pool(name="vsw", bufs=2))
        rope_pool = stack.enter_context(tc.tile_pool(name="rope", bufs=2))
        fin_pool = stack.enter_context(tc.tile_pool(name="fin", bufs=2))
        yt_pool = stack.enter_context(tc.tile_pool(name="yt", bufs=2))

        # PSUM: mm(3) shared by proj/Wo + sps(3) + aps(2) = 8 banks
        mm_ps = stack.enter_context(tc.tile_pool(name="mm_ps", bufs=3, space="PSUM"))
        s_ps = stack.enter_context(tc.tile_pool(name="s_ps", bufs=3, space="PSUM"))
        a_ps = stack.enter_context(tc.tile_pool(name="a_ps", bufs=2, space="PSUM"))

        # weights + first x block, interleaved so matmul i can start early;
        # the first stripe ships its V-columns first (first PE matmul)
        nc.sync.dma_start(wqkv[0][:, (G + 1) * DH:], wqkv_d[0:128, (G + 1) * DH:])
        nc.sync.dma_start(xts[0][0][:], xt_d[0:128, 0:SB])
        nc.sync.dma_start(wqkv[0][:, 0:(G + 1) * DH], wqkv_d[0:128, 0:(G + 1) * DH])
        for i in range(1, ND):
            nc.sync.dma_start(wqkv[i][:], wqkv_d[128 * i:128 * (i + 1), :])
            nc.sync.dma_start(xts[0][i][:], xt_d[128 * i:128 * (i + 1), 0:SB])
        # first rope-table chunk right behind (gates the first rope_evict);
        # the rest follows in proj_block(0)
        nc.sync.dma_start(cost[0][:], cos_d[:, 0:SB])
        nc.sync.dma_start(sint[0][:], sin_d[:, 0:SB])
        # small tensors: needed only from attention(0) on
        nc.sync.dma_start(mask[:], msk_d[:])

        def rope_evict(ps, out_tile, sb):
            # Cross-half (rotate-half) reads live on the PSUM operand: the
            # BIR verifier only requires equal base partitions when BOTH
            # tensor_tensor inputs are in SBUF. Muls read PSUM -> DVE; the
            # final add is SBUF-only and partition-aligned -> Pool engine.
            ts_ = rope_pool.tile([128, SB], f32, name="tsin", tag="tsin")
            tcs = rope_pool.tile([128, SB], f32, name="tcos", tag="tcos")
            nc.vector.tensor_mul(ts_[0:64, :], ps[64:128, :], sint[sb][0:64, :])
            nc.vector.tensor_mul(ts_[64:128, :], ps[0:64, :], sint[sb][64:128, :])
            nc.vector.tensor_mul(tcs[:], ps[:], cost[sb][:, :])
            nc.gpsimd.tensor_add(out_tile[:], tcs[:], ts_[:])

        def proj_block(sb):
            c0 = SB * sb
            xt = xts[sb]
            KO, VO = G * DH, (G + 1) * DH  # column offsets of Wk / Wv

            def mm_group(lo, interleaved=()):
                ps = mm_ps.tile([128, SB], f32, name="pp", tag="pp")
                for i in range(ND):
                    nc.tensor.matmul(
                        ps[:], wqkv[i][:, lo:lo + DH], xt[i][:],
                        start=(i == 0), stop=(i == ND - 1),
                        skip_group_check=bool(interleaved))
                return ps

            if sb == 0:
                # x0 tiles arrive at DMA pace: interleave all 6 psum groups
                # by contraction index so the PE rides the DMA wave,
                # borrowing the (not yet used) attention PSUM banks
                pools = [mm_ps, mm_ps, mm_ps, s_ps, s_ps, a_ps]
                tags = ["pp", "pp", "pp", "sps", "sps", "aps"]
                offsets = [VO, KO, 0, DH, 2 * DH, 3 * DH]
                pss = [pool.tile([128, SB], f32, name="pp", tag=t)
                       for pool, t in zip(pools, tags)]
                for i in range(ND):
                    for ps, lo in zip(pss, offsets):
                        nc.tensor.matmul(
                            ps[:], wqkv[i][:, lo:lo + DH], xt[i][:],
                            start=(i == 0), stop=(i == ND - 1),
                            skip_group_check=True)
            def v_evict_transpose(vps):
                # chunked evict + immediate transpose DMA: each vsb[] tile
                # becomes ready as early as possible (the list scheduler
                # slots SP work by readiness, and the PV matmuls need these)
                v_sb = vsbw_pool.tile([128, SB], bf16, name="vsb", tag="vsb")
                for c in range(SB // 128):
                    cc = slice(128 * c, 128 * (c + 1))
                    nc.scalar.copy(v_sb[:, cc], vps[:, cc])
                    nc.sync.dma_start_transpose(vsb[4 * sb + c][:], v_sb[:, cc])

            if sb == 0:
                vps, kps, qps0, qps1, qps2, qps3 = pss
                v_evict_transpose(vps)
                rope_evict(kps, krt[sb], sb)
                rope_evict(qps0, qrt[0][sb], sb)
                rope_evict(qps1, qrt[1][sb], sb)
                rope_evict(qps2, qrt[2][sb], sb)
                rope_evict(qps3, qrt[3][sb], sb)
            else:
                vps = mm_group(VO)
                kps = mm_group(KO)
                v_evict_transpose(vps)
                qps0 = mm_group(0)
                rope_evict(kps, krt[sb], sb)
                qps1 = mm_group(DH)
                rope_evict(qps0, qrt[0][sb], sb)
                qps2 = mm_group(2 * DH)
                rope_evict(qps1, qrt[1][sb], sb)
                qps3 = mm_group(3 * DH)
                rope_evict(qps2, qrt[2][sb], sb)
                rope_evict(qps3, qrt[3][sb], sb)

            # DMA schedule: tables/weights for upcoming phases, next x block
            if sb + 1 < NSB:
                for i in range(ND):
                    nc.sync.dma_start(
                        xts[sb + 1][i][:],
                        xt_d[128 * i:128 * (i + 1), SB * (sb + 1):SB * (sb + 2)])
                    if sb == 0 and i == 7:
                        # rope tables for block 1 slip in mid-bulk: early
                        # enough for proj(1)'s K rope, without starving the
                        # PE of x1 tiles
                        nc.sync.dma_start(cost[1][:], cos_d[:, SB:2 * SB])
                        nc.sync.dma_start(sint[1][:], sin_d[:, SB:2 * SB])
            if sb == 0:
                for s in range(2, NSB):
                    nc.sync.dma_start(cost[s][:], cos_d[:, SB * s:SB * (s + 1)])
                    nc.sync.dma_start(sint[s][:], sin_d[:, SB * s:SB * (s + 1)])
            if sb == 1:
                for h in range(G):
                    nc.sync.dma_start(wot[h][:], wot_d[128 * h:128 * (h + 1), :])

        # deferred finalize steps, drained one per attention block
        fin_steps = []

        def drain_one():
            if fin_steps:
                step = fin_steps.pop(0)
                if step is not None:
                    step()

        def drain_all():
            while fin_steps:
                step = fin_steps.pop(0)
                if step is not None:
                    step()

        def finalize_lazy(h, qb, aps, dacc, daccp):
            """Denominator partition-reduce (Pool) -> reciprocal -> normalize,
            as deferred steps.

            Each step is drained with spacing so cross-engine latency
            (DVE->Pool->DVE) never stalls the PE. partition_all_reduce leaves
            the sum on ALL partitions, so no broadcast step is needed.
            """
            from concourse import bass_isa
            st = {}

            def s0():
                nc.gpsimd.tensor_add(daccp[:], daccp[:], dacc[:])

            def s1():
                dall = fin_pool.tile([128, SB], f32, name="dall", tag="dall")
                nc.gpsimd.partition_all_reduce(
                    dall[:], daccp[:], channels=128,
                    reduce_op=bass_isa.ReduceOp.add)
                st["dall"] = dall

            def s2():
                rbc = fin_pool.tile([128, SB], f32, name="rbc", tag="rbc")
                nc.vector.reciprocal(rbc[:], st["dall"][:])
                st["rbc"] = rbc

            def s3():
                nc.vector.tensor_mul(a_t[h][qb][:], aps[:], st["rbc"][:])

            fin_steps.extend([s0, s1, None, s2, None, s3])

        def attn_head(h, qb, defer_finalize=False):
            """scores -> exp -> (mask) -> dacc accumulate -> PV accumulate.

            PE emission has one-block lookahead: scores(kb+1) before PV(kb).
            """
            nkb = 4 * qb + 4
            aps = a_ps.tile([128, SB], f32, name="aps", tag="aps")
            # two partial denominator accumulators: even k-blocks on DVE,
            # odd on Pool (both SBUF-only, legal for GPSIMD); combined at
            # the end on Pool
            dacc = dacc_pool.tile([128, SB], bf16, name="dacc", tag="dacc")
            daccp = dacc_pool.tile([128, SB], bf16, name="daccp", tag="daccp")
            pend = []  # (kb, p, qoff, w), lookahead-2 queue
            for kb in range(nkb):
                j = kb - 4 * qb
                qoff = 128 * j if j > 0 else 0
                w = SB - qoff
                sps = s_ps.tile([128, SB], f32, name="sps", tag="sps")
                nc.tensor.matmul(
                    sps[:, 0:w], krt[kb // 4][:, 128 * (kb % 4):128 * (kb % 4 + 1)],
                    qrt[h][qb][:, qoff:SB],
                    start=True, stop=True, skip_group_check=True)
                p = p_pool.tile([128, SB], bf16, name="p", tag="p")
                nc.scalar.activation(p[:, 0:w], sps[:, 0:w], AF.Exp, scale=SCALE)
                if j >= 0:
                    nc.vector.tensor_mul(p[:, 0:w], p[:, 0:w], mask[:, 0:w])
                eng, acc = ((nc.vector, dacc) if kb % 2 == 0
                            else (nc.gpsimd, daccp))
                if kb < 2:
                    if qoff:
                        eng.memset(acc[:, 0:qoff], 0.0)
                    eng.tensor_copy(acc[:, qoff:SB], p[:, 0:w])
                else:
                    eng.tensor_add(acc[:, qoff:SB], acc[:, qoff:SB], p[:, 0:w])
                if len(pend) == 2:
                    pkb, pp, pqoff, pw = pend.pop(0)
                    nc.tensor.matmul(
                        aps[:, pqoff:SB], vsb[pkb][:], pp[:, 0:pw],
                        start=(pkb == 0), stop=False, skip_group_check=True)
                pend.append((kb, p, qoff, w))
                drain_one()
            while pend:
                pkb, pp, pqoff, pw = pend.pop(0)
                nc.tensor.matmul(
                    aps[:, pqoff:SB], vsb[pkb][:], pp[:, 0:pw],
                    start=(pkb == 0), stop=(not pend), skip_group_check=True)
            if defer_finalize:
                return aps, dacc, daccp
            finalize_lazy(h, qb, aps, dacc, daccp)
            return None

        def wo_quarter(qb, c, final=False, evict_act=False):
            """y rows [128c'..] for query block qb, quarter c: 4 eb psums.

            Mid-attention quarters evict on DVE (Act runs at exactly PE pace
            there with exp work); endgame quarters evict on Act (idle after
            the last exp, and DVE holds the finalize chains). The final
            quarter DMAs per-eb to shorten the tail.
            """
            yt = yt_pool.tile([128, D], bf16, name="yt", tag="yt")
            sb128 = 4 * qb + c
            for eb in range(NSB):
                yp = mm_ps.tile([128, SB], f32, name="pp", tag="pp")
                for h in range(G):
                    nc.tensor.matmul(
                        yp[:], a_t[h][qb][:, 128 * c:128 * (c + 1)],
                        wot[h][:, SB * eb:SB * (eb + 1)],
                        start=(h == 0), stop=(h == G - 1))
                if final:
                    # per-eb evict+DMA pipeline to shorten the tail
                    nc.scalar.copy(yt[:, SB * eb:SB * (eb + 1)], yp[:])
                    nc.sync.dma_start(
                        y_d[128 * sb128:128 * (sb128 + 1),
                            SB * eb:SB * (eb + 1)],
                        yt[:, SB * eb:SB * (eb + 1)])
                elif evict_act:
                    nc.scalar.copy(yt[:, SB * eb:SB * (eb + 1)], yp[:])
                else:
                    nc.vector.tensor_copy(yt[:, SB * eb:SB * (eb + 1)], yp[:])
            if not final:
                nc.sync.dma_start(
                    y_d[128 * sb128:128 * (sb128 + 1), :], yt[:])

        last = None
        for sb in range(NSB):
            proj_block(sb)
            drain_all()  # a_t[*][sb-1] must be written before wo_quarter reads
            for h in range(G):
                if sb > 0:
                    wo_quarter(sb - 1, h)
                last = attn_head(h, sb,
                                 defer_finalize=(sb == NSB - 1 and h == G - 1))
        drain_all()

        # last head's finalize is latency-critical (it gates the final Wo
        # chunk): run it at 128-column granularity, pipelined against the
        # final Wo quarters
        from concourse import bass_isa
        aps, dacc, daccp = last
        qb = NSB - 1
        dall = fin_pool.tile([128, SB], f32, name="dall", tag="dall")
        for c in range(G):
            cs = slice(128 * c, 128 * (c + 1))
            nc.gpsimd.tensor_add(daccp[:, cs], daccp[:, cs], dacc[:, cs])
            nc.gpsimd.partition_all_reduce(
                dall[:, cs], daccp[:, cs], channels=128,
                reduce_op=bass_isa.ReduceOp.add)
            rbc = fin_pool.tile([128, 128], f32, name="rbcc", tag="rbcc")
            nc.vector.reciprocal(rbc[:], dall[:, cs])
            nc.vector.tensor_mul(a_t[G - 1][qb][:, cs], aps[:, cs], rbc[:])
            if c == 0:
                # wave-open: the h0..h2 accumulation of the first quarter
                # does not depend on the chunk finalize above, so it fills
                # the cross-engine chain latency on the PE
                yt = yt_pool.tile([128, D], bf16, name="yt", tag="yt")
                open_ps = []
                for eb in range(3):
                    yp = mm_ps.tile([128, SB], f32, name="pp", tag="pp")
                    for h in range(G - 1):
                        nc.tensor.matmul(
                            yp[:], a_t[h][qb][:, cs],
                            wot[h][:, SB * eb:SB * (eb + 1)],
                            start=(h == 0), stop=False, skip_group_check=True)
                    open_ps.append(yp)
                for eb in range(3):
                    nc.tensor.matmul(
                        open_ps[eb][:], a_t[G - 1][qb][:, cs],
                        wot[G - 1][:, SB * eb:SB * (eb + 1)],
                        start=False, stop=True, skip_group_check=True)
                    nc.scalar.copy(
                        yt[:, SB * eb:SB * (eb + 1)], open_ps[eb][:])
                yp = mm_ps.tile([128, SB], f32, name="pp", tag="pp")
                for h in range(G):
                    nc.tensor.matmul(
                        yp[:], a_t[h][qb][:, cs], wot[h][:, 3 * SB:4 * SB],
                        start=(h == 0), stop=(h == G - 1))
                nc.scalar.copy(yt[:, 3 * SB:4 * SB], yp[:])
                nc.sync.dma_start(
                    y_d[128 * 4 * qb:128 * (4 * qb + 1), :], yt[:])
            else:
                wo_quarter(qb, c, final=(c == G - 1), evict_act=True)

    nc.compile()
    return nc


def _rope_tables():
    inv = 1.0 / (ROPE_THETA ** (np.arange(0, DH, 2, dtype=np.float64) / DH))
    pos = np.arange(S, dtype=np.float64)
    theta = np.concatenate([np.outer(pos, inv)] * 2, axis=1)  # [S, DH]
    cosT = np.cos(theta).T.astype(np.float32)                 # [DH, S]
    sinT = np.sin(theta).T.astype(np.float32)
    sints = np.concatenate([-sinT[:64], sinT[64:]], axis=0)
    bf16 = ml_dtypes.bfloat16
    return (np.ascontiguousarray(cosT.astype(bf16)),
            np.ascontiguousarray(sints.astype(bf16)))


def build_in_maps(x, Wq, Wk, Wv, Wo):
    bf16 = ml_dtypes.bfloat16
    x = np.asarray(x, np.float32)
    Wq = np.asarray(Wq, np.float32)
    Wk = np.asarray(Wk, np.float32)
    Wv = np.asarray(Wv, np.float32)
    Wo = np.asarray(Wo, np.float32)
    cosT, sints = _rope_tables()
    r_ = np.arange(128)[:, None]
    c_ = np.arange(SB)[None, :]
    mask = (c_ >= r_).astype(np.float32).astype(bf16)
    xt_b = [np.ascontiguousarray(x[b].T.astype(bf16)) for b in range(B)]
    in_maps = []
    for core in range(NCORES):
        b, g = divmod(core, HKV)
        wqkv = np.concatenate([
            Wq[G * DH * g:G * DH * (g + 1)].T,
            Wk[DH * g:DH * (g + 1)].T,
            Wv[DH * g:DH * (g + 1)].T,
        ], axis=1).astype(bf16)
        in_maps.append({
            "xt": xt_b[b],
            "wqkv": np.ascontiguousarray(wqkv),
            "wot": np.ascontiguousarray(
                Wo[:, G * DH * g:G * DH * (g + 1)].T.astype(bf16)),
            "cost": cosT,
            "sints": sints,
            "masks": mask,
        })
    return in_maps


def get_nc():
    if "nc" not in _CACHE:
        _CACHE["nc"] = _build_nc()
    return _CACHE["nc"]


def kernel(x, Wq, Wk, Wv, Wo):
    from concourse.bass_utils import run_bass_kernel_spmd

    nc = get_nc()
    in_maps = build_in_maps(x, Wq, Wk, Wv, Wo)
    res = run_bass_kernel_spmd(nc, in_maps, list(range(NCORES)))
    parts = [res.results[c]["y"].astype(np.float32) for c in range(NCORES)]
    y = np.stack([
        parts[0] + parts[1] + parts[2] + parts[3],
        parts[4] + parts[5] + parts[6] + parts[7],
    ]).astype(np.float32)
    return y


# revision 73
# speedup vs baseline: 1.0609x; 1.0609x over previous
"""GQA (grouped-query attention) Trainium2 kernel, 8-core SPMD.

Sharding: TP=4 over kv-heads x DP=2 over batch  (core = b*4 + g).
Each core computes, for its batch b and kv-head g (q-heads 4g..4g+3):
  QKV projections -> RoPE -> causal softmax(QK^T)V -> partial x@Wo
entirely in transposed layout (feature dim on SBUF partitions), then the
host sums the 4 partial Wo outputs per batch (the TP all-reduce).

Single fused pipeline over the 4 query/sequence blocks of 512:
  proj(sb) -> attention(qb=sb) -> Wo-chunk(qb-1) quarters interleaved
between attention heads so the PE never drains.

Dataflow notes:
 - all matmul operands in bf16 (full PE rate, halves HBM traffic)
 - causal structure: strictly-upper k-blocks skipped; the 4 diagonal
   k-blocks use narrowed moving operands (512/384/256/128 queries) and a
   multiplicative triangular mask after exp
 - softmax runs in S^T[k,q] orientation; denominators: exp tiles are
   accumulated in bf16 into two partial tiles (even k-blocks on the vector
   engine, odd on GPSIMD), reduced across partitions per (head, q-block)
   with gpsimd.partition_all_reduce — the PE does no softmax bookkeeping
 - no max-subtraction: scores are bounded (~+-5) for this problem size
 - V^T -> V[k,dv] reorientation via DMA transpose (16-bit), not the PE
"""

import math
import sys

import numpy as np

if "/opt/trn_rl_repo" not in sys.path:
    sys.path.insert(0, "/opt/trn_rl_repo")

import ml_dtypes

B, S, D = 2, 2048, 2048
HQ, HKV, DH = 16, 4, 128
G = HQ // HKV            # q-heads per kv-head = 4
NCORES = 8
ROPE_THETA = 10000.0
SCALE = 1.0 / math.sqrt(DH)

SB = 512                 # wide column block (moving operand)
NSB = S // SB            # 4
ND = D // 128            # 16 contraction tiles
NKB = S // 128           # 16 key blocks

_CACHE = {}


def _build_nc():
    import concourse.bass as bass
    import concourse.mybir as mybir
    import concourse.tile as tile
    from concourse import bacc

    f32 = mybir.dt.float32
    bf16 = mybir.dt.bfloat16
    AF = mybir.ActivationFunctionType

    nc = bacc.Bacc(
        trn_type="TRN2", target_bir_lowering=False, debug=False,
        num_devices=NCORES,
    )

    xt_d = nc.dram_tensor("xt", [D, S], bf16, kind="ExternalInput").ap()
    wqkv_d = nc.dram_tensor("wqkv", [D, (G + 2) * DH], bf16, kind="ExternalInput").ap()
    wot_d = nc.dram_tensor("wot", [G * DH, D], bf16, kind="ExternalInput").ap()
    cos_d = nc.dram_tensor("cost", [DH, S], bf16, kind="ExternalInput").ap()
    sin_d = nc.dram_tensor("sints", [DH, S], bf16, kind="ExternalInput").ap()
    msk_d = nc.dram_tensor("masks", [128, SB], bf16, kind="ExternalInput").ap()
    y_d = nc.dram_tensor("y", [S, D], bf16, kind="ExternalOutput").ap()

    from contextlib import ExitStack

    with tile.TileContext(nc) as tc, ExitStack() as stack, \
            nc.allow_low_precision(reason="bf16 matmul operands"):
        persist = stack.enter_context(tc.tile_pool(name="persist", bufs=1))

        # persistent SBUF tensors (per-block tiles: no reliance on sub-tile
        # dependency tracking)
        xts = [[persist.tile([128, SB], bf16, name=f"x{s}_{i}", tag=f"x{s}_{i}")
                for i in range(ND)] for s in range(NSB)]
        wqkv = [persist.tile([128, (G + 2) * DH], bf16, name=f"w{i}", tag=f"w{i}")
                for i in range(ND)]
        wot = [persist.tile([128, D], bf16, name=f"wo{h}", tag=f"wo{h}")
               for h in range(G)]
        qrt = [[persist.tile([128, SB], bf16, name=f"q{h}_{s}", tag=f"q{h}_{s}")
                for s in range(NSB)] for h in range(G)]
        krt = [persist.tile([128, SB], bf16, name=f"k{s}", tag=f"k{s}")
               for s in range(NSB)]
        vsb = [persist.tile([128, DH], bf16, name=f"v{k}", tag=f"v{k}")
               for k in range(NKB)]
        a_t = [[persist.tile([128, SB], bf16, name=f"a{h}_{s}", tag=f"a{h}_{s}")
                for s in range(NSB)] for h in range(G)]
        cost = [persist.tile([128, SB], bf16, name=f"cost{s}", tag=f"cost{s}")
                for s in range(NSB)]
        sint = [persist.tile([128, SB], bf16, name=f"sint{s}", tag=f"sint{s}")
                for s in range(NSB)]
        mask = persist.tile([128, SB], bf16, name="mask", tag="mask")

        # working rings (SBUF)
        p_pool = stack.enter_context(tc.tile_pool(name="pp", bufs=4))
        dacc_pool = stack.enter_context(tc.tile_pool(name="dap", bufs=2))
        vsbw_pool = stack.enter_context(tc.tile_pool(name="vsw", bufs=2))
        rope_pool = stack.enter_context(tc.tile_pool(name="rope", bufs=2))
        fin_pool = stack.enter_context(tc.tile_pool(name="fin", bufs=2))
        yt_pool = stack.enter_context(tc.tile_pool(name="yt", bufs=2))

        # PSUM: mm(3) shared by proj/Wo + sps(3) + aps(2) = 8 banks
        mm_ps = stack.enter_context(tc.tile_pool(name="mm_ps", bufs=3, space="PSUM"))
        s_ps = stack.enter_context(tc.tile_pool(name="s_ps", bufs=3, space="PSUM"))
        a_ps = stack.enter_context(tc.tile_pool(name="a_ps", bufs=2, space="PSUM"))

        # weights + first x block, interleaved so matmul i can start early;
        # the first stripe ships its V-columns first (first PE matmul)
        nc.sync.dma_start(wqkv[0][:, (G + 1) * DH:], wqkv_d[0:128, (G + 1) * DH:])
        nc.sync.dma_start(xts[0][0][:], xt_d[0:128, 0:SB])
        nc.sync.dma_start(wqkv[0][:, 0:(G + 1) * DH], wqkv_d[0:128, 0:(G + 1) * DH])
        for i in range(1, ND):
            nc.sync.dma_start(wqkv[i][:], wqkv_d[128 * i:128 * (i + 1), :])
            nc.sync.dma_start(xts[0][i][:], xt_d[128 * i:128 * (i + 1), 0:SB])
        # first rope-table chunk right behind (gates the first rope_evict);
        # the rest follows in proj_block(0)
        nc.sync.dma_start(cost[0][:], cos_d[:, 0:SB])
        nc.sync.dma_start(sint[0][:], sin_d[:, 0:SB])
        # small tensors: needed only from attention(0) on
        nc.sync.dma_start(mask[:], msk_d[:])

        def rope_evict(ps, out_tile, sb):
            # Cross-half (rotate-half) reads live on the PSUM operand: the
            # BIR verifier only requires equal base partitions when BOTH
            # tensor_tensor inputs are in SBUF. Muls read PSUM -> DVE; the
            # final add is SBUF-only and partition-aligned -> Pool engine.
            ts_ = rope_pool.tile([128, SB], f32, name="tsin", tag="tsin")
            tcs = rope_pool.tile([128, SB], f32, name="tcos", tag="tcos")
            nc.vector.tensor_mul(ts_[0:64, :], ps[64:128, :], sint[sb][0:64, :])
            nc.vector.tensor_mul(ts_[64:128, :], ps[0:64, :], sint[sb][64:128, :])
            nc.vector.tensor_mul(tcs[:], ps[:], cost[sb][:, :])
            nc.gpsimd.tensor_add(out_tile[:], tcs[:], ts_[:])

        def proj_block(sb):
            c0 = SB * sb
            xt = xts[sb]
            KO, VO = G * DH, (G + 1) * DH  # column offsets of Wk / Wv

            def mm_group(lo, interleaved=()):
                ps = mm_ps.tile([128, SB], f32, name="pp", tag="pp")
                for i in range(ND):
                    nc.tensor.matmul(
                        ps[:], wqkv[i][:, lo:lo + DH], xt[i][:],
                        start=(i == 0), stop=(i == ND - 1),
                        skip_group_check=bool(interleaved))
                return ps

            if sb == 0:
                # x0 tiles arrive at DMA pace: interleave all 6 psum groups
                # by contraction index so the PE rides the DMA wave,
                # borrowing the (not yet used) attention PSUM banks
                pools = [mm_ps, mm_ps, mm_ps, s_ps, s_ps, a_ps]
                tags = ["pp", "pp", "pp", "sps", "sps", "aps"]
                offsets = [VO, KO, 0, DH, 2 * DH, 3 * DH]
                pss = [pool.tile([128, SB], f32, name="pp", tag=t)
                       for pool, t in zip(pools, tags)]
                for i in range(ND):
                    for ps, lo in zip(pss, offsets):
                        nc.tensor.matmul(
                            ps[:], wqkv[i][:, lo:lo + DH], xt[i][:],
                            start=(i == 0), stop=(i == ND - 1),
                            skip_group_check=True)
            def v_evict_transpose(vps):
                # chunked evict + immediate transpose DMA: each vsb[] tile
                # becomes ready as early as possible (the list scheduler
                # slots SP work by readiness, and the PV matmuls need these)
                v_sb = vsbw_pool.tile([128, SB], bf16, name="vsb", tag="vsb")
                for c in range(SB // 128):
                    cc = slice(128 * c, 128 * (c + 1))
                    nc.scalar.copy(v_sb[:, cc], vps[:, cc])
                    nc.sync.dma_start_transpose(vsb[4 * sb + c][:], v_sb[:, cc])

            if sb == 0:
                vps, kps, qps0, qps1, qps2, qps3 = pss
                v_evict_transpose(vps)
                rope_evict(kps, krt[sb], sb)
                rope_evict(qps0, qrt[0][sb], sb)
                rope_evict(qps1, qrt[1][sb], sb)
                rope_evict(qps2, qrt[2][sb], sb)
                rope_evict(qps3, qrt[3][sb], sb)
            else:
                vps = mm_group(VO)
                kps = mm_group(KO)
                v_evict_transpose(vps)
                qps0 = mm_group(0)
                rope_evict(kps, krt[sb], sb)
                qps1 = mm_group(DH)
                rope_evict(qps0, qrt[0][sb], sb)
                qps2 = mm_group(2 * DH)
                rope_evict(qps1, qrt[1][sb], sb)
                qps3 = mm_group(3 * DH)
                rope_evict(qps2, qrt[2][sb], sb)
                rope_evict(qps3, qrt[3][sb], sb)

            # DMA schedule: tables/weights for upcoming phases, next x block
            if sb + 1 < NSB:
                for i in range(ND):
                    nc.sync.dma_start(
                        xts[sb + 1][i][:],
                        xt_d[128 * i:128 * (i + 1), SB * (sb + 1):SB * (sb + 2)])
                    if sb == 0 and i == 7:
                        # rope tables for block 1 slip in mid-bulk: early
                        # enough for proj(1)'s K rope, without starving the
                        # PE of x1 tiles
                        nc.sync.dma_start(cost[1][:], cos_d[:, SB:2 * SB])
                        nc.sync.dma_start(sint[1][:], sin_d[:, SB:2 * SB])
            if sb == 0:
                for s in range(2, NSB):
                    nc.sync.dma_start(cost[s][:], cos_d[:, SB * s:SB * (s + 1)])
                    nc.sync.dma_start(sint[s][:], sin_d[:, SB * s:SB * (s + 1)])
            if sb == 1:
                for h in range(G):
                    nc.sync.dma_start(wot[h][:], wot_d[128 * h:128 * (h + 1), :])

        # deferred finalize steps, drained one per attention block
        fin_steps = []

        def drain_one():
            if fin_steps:
                step = fin_steps.pop(0)
                if step is not None:
                    step()

        def drain_all():
            while fin_steps:
                step = fin_steps.pop(0)
                if step is not None:
                    step()

        def finalize_lazy(h, qb, aps, dacc, daccp):
            """Denominator partition-reduce (Pool) -> reciprocal -> normalize,
            as deferred steps.

            Each step is drained with spacing so cross-engine latency
            (DVE->Pool->DVE) never stalls the PE. partition_all_reduce leaves
            the sum on ALL partitions, so no broadcast step is needed.
            """
            from concourse import bass_isa
            st = {}

            def s0():
                nc.gpsimd.tensor_add(daccp[:], daccp[:], dacc[:])

            def s1():
                dall = fin_pool.tile([128, SB], f32, name="dall", tag="dall")
                nc.gpsimd.partition_all_reduce(
                    dall[:], daccp[:], channels=128,
                    reduce_op=bass_isa.ReduceOp.add)
                st["dall"] = dall

            def s2():
                rbc = fin_pool.tile([128, SB], f32, name="rbc", tag="rbc")
                nc.vector.reciprocal(rbc[:], st["dall"][:])
                st["rbc"] = rbc

            def s3():
                nc.vector.tensor_mul(a_t[h][qb][:], aps[:], st["rbc"][:])

            fin_steps.extend([s0, s1, None, s2, None, s3])

        def attn_head(h, qb, defer_finalize=False):
            """scores -> exp -> (mask) -> dacc accumulate -> PV accumulate.

            PE emission has one-block lookahead: scores(kb+1) before PV(kb).
            """
            nkb = 4 * qb + 4
            aps = a_ps.tile([128, SB], f32, name="aps", tag="aps")
            # two partial denominator accumulators: even k-blocks on DVE,
            # odd on Pool (both SBUF-only, legal for GPSIMD); combined at
            # the end on Pool
            dacc = dacc_pool.tile([128, SB], bf16, name="dacc", tag="dacc")
            daccp = dacc_pool.tile([128, SB], bf16, name="daccp", tag="daccp")
            pend = []  # (kb, p, qoff, w), lookahead-2 queue
            lookahead = 2
            for kb in range(nkb):
                j = kb - 4 * qb
                qoff = 128 * j if j > 0 else 0
                w = SB - qoff
                sps = s_ps.tile([128, SB], f32, name="sps", tag="sps")
                nc.tensor.matmul(
                    sps[:, 0:w], krt[kb // 4][:, 128 * (kb % 4):128 * (kb % 4 + 1)],
                    qrt[h][qb][:, qoff:SB],
                    start=True, stop=True, skip_group_check=True)
                p = p_pool.tile([128, SB], bf16, name="p", tag="p")
                nc.scalar.activation(p[:, 0:w], sps[:, 0:w], AF.Exp, scale=SCALE)
                if j >= 0:
                    nc.vector.tensor_mul(p[:, 0:w], p[:, 0:w], mask[:, 0:w])
                eng, acc = ((nc.vector, dacc) if kb % 2 == 0
                            else (nc.gpsimd, daccp))
                if kb < 2:
                    if qoff:
                        eng.memset(acc[:, 0:qoff], 0.0)
                    eng.tensor_copy(acc[:, qoff:SB], p[:, 0:w])
                else:
                    eng.tensor_add(acc[:, qoff:SB], acc[:, qoff:SB], p[:, 0:w])
                if len(pend) == lookahead:
                    pkb, pp, pqoff, pw = pend.pop(0)
                    nc.tensor.matmul(
                        aps[:, pqoff:SB], vsb[pkb][:], pp[:, 0:pw],
                        start=(pkb == 0), stop=False, skip_group_check=True)
                pend.append((kb, p, qoff, w))
                drain_one()
            while pend:
                pkb, pp, pqoff, pw = pend.pop(0)
                nc.tensor.matmul(
                    aps[:, pqoff:SB], vsb[pkb][:], pp[:, 0:pw],
                    start=(pkb == 0), stop=(not pend), skip_group_check=True)
            if defer_finalize:
                return aps, dacc, daccp
            finalize_lazy(h, qb, aps, dacc, daccp)
            return None

        def wo_quarter(qb, c, final=False, evict_act=False):
            """y rows [128c'..] for query block qb, quarter c: 4 eb psums.

            Mid-attention quarters evict on DVE (Act runs at exactly PE pace
            there with exp work); endgame quarters evict on Act (idle after
            the last exp, and DVE holds the finalize chains). The final
            quarter DMAs per-eb to shorten the tail.
            """
            yt = yt_pool.tile([128, D], bf16, name="yt", tag="yt")
            sb128 = 4 * qb + c
            for eb in range(NSB):
                yp = mm_ps.tile([128, SB], f32, name="pp", tag="pp")
                for h in range(G):
                    nc.tensor.matmul(
                        yp[:], a_t[h][qb][:, 128 * c:128 * (c + 1)],
                        wot[h][:, SB * eb:SB * (eb + 1)],
                        start=(h == 0), stop=(h == G - 1))
                if final:
                    # per-eb evict+DMA pipeline to shorten the tail
                    nc.scalar.copy(yt[:, SB * eb:SB * (eb + 1)], yp[:])
                    nc.sync.dma_start(
                        y_d[128 * sb128:128 * (sb128 + 1),
                            SB * eb:SB * (eb + 1)],
                        yt[:, SB * eb:SB * (eb + 1)])
                elif evict_act:
                    nc.scalar.copy(yt[:, SB * eb:SB * (eb + 1)], yp[:])
                else:
                    nc.vector.tensor_copy(yt[:, SB * eb:SB * (eb + 1)], yp[:])
            if not final:
                nc.sync.dma_start(
                    y_d[128 * sb128:128 * (sb128 + 1), :], yt[:])

        last = None
        for sb in range(NSB):
            proj_block(sb)
            drain_all()  # a_t[*][sb-1] must be written before wo_quarter reads
            for h in range(G):
                if sb > 0:
                    wo_quarter(sb - 1, h)
                last = attn_head(h, sb,
                                 defer_finalize=(sb == NSB - 1 and h == G - 1))
        drain_all()

        # last head's finalize is latency-critical (it gates the final Wo
        # chunk): run it at 128-column granularity, pipelined against the
        # final Wo quarters
        from concourse import bass_isa
        aps, dacc, daccp = last
        qb = NSB - 1
        dall = fin_pool.tile([128, SB], f32, name="dall", tag="dall")
        for c in range(G):
            cs = slice(128 * c, 128 * (c + 1))
            nc.gpsimd.tensor_add(daccp[:, cs], daccp[:, cs], dacc[:, cs])
            nc.gpsimd.partition_all_reduce(
                dall[:, cs], daccp[:, cs], channels=128,
                reduce_op=bass_isa.ReduceOp.add)
            rbc = fin_pool.tile([128, 128], f32, name="rbcc", tag="rbcc")
            nc.vector.reciprocal(rbc[:], dall[:, cs])
            nc.vector.tensor_mul(a_t[G - 1][qb][:, cs], aps[:, cs], rbc[:])
            if c == 0:
                # wave-open: the h0..h2 accumulation of the first quarter
                # does not depend on the chunk finalize above, so it fills
                # the cross-engine chain latency on the PE
                yt = yt_pool.tile([128, D], bf16, name="yt", tag="yt")
                open_ps = []
                for eb in range(3):
                    yp = mm_ps.tile([128, SB], f32, name="pp", tag="pp")
                    for h in range(G - 1):
                        nc.tensor.matmul(
                            yp[:], a_t[h][qb][:, cs],
                            wot[h][:, SB * eb:SB * (eb + 1)],
                            start=(h == 0), stop=False, skip_group_check=True)
                    open_ps.append(yp)
                for eb in range(3):
                    nc.tensor.matmul(
                        open_ps[eb][:], a_t[G - 1][qb][:, cs],
                        wot[G - 1][:, SB * eb:SB * (eb + 1)],
                        start=False, stop=True, skip_group_check=True)
                    nc.scalar.copy(
                        yt[:, SB * eb:SB * (eb + 1)], open_ps[eb][:])
                yp = mm_ps.tile([128, SB], f32, name="pp", tag="pp")
                for h in range(G):
                    nc.tensor.matmul(
                        yp[:], a_t[h][qb][:, cs], wot[h][:, 3 * SB:4 * SB],
                        start=(h == 0), stop=(h == G - 1))
                nc.scalar.copy(yt[:, 3 * SB:4 * SB], yp[:])
                nc.sync.dma_start(
                    y_d[128 * 4 * qb:128 * (4 * qb + 1), :], yt[:])
            else:
                wo_quarter(qb, c, final=(c == G - 1), evict_act=True)

    nc.compile()
    return nc


def _rope_tables():
    inv = 1.0 / (ROPE_THETA ** (np.arange(0, DH, 2, dtype=np.float64) / DH))
    pos = np.arange(S, dtype=np.float64)
    theta = np.concatenate([np.outer(pos, inv)] * 2, axis=1)  # [S, DH]
    cosT = np.cos(theta).T.astype(np.float32)                 # [DH, S]
    sinT = np.sin(theta).T.astype(np.float32)
    sints = np.concatenate([-sinT[:64], sinT[64:]], axis=0)
    bf16 = ml_dtypes.bfloat16
    return (np.ascontiguousarray(cosT.astype(bf16)),
            np.ascontiguousarray(sints.astype(bf16)))


def build_in_maps(x, Wq, Wk, Wv, Wo):
    bf16 = ml_dtypes.bfloat16
    x = np.asarray(x, np.float32)
    Wq = np.asarray(Wq, np.float32)
    Wk = np.asarray(Wk, np.float32)
    Wv = np.asarray(Wv, np.float32)
    Wo = np.asarray(Wo, np.float32)
    cosT, sints = _rope_tables()
    r_ = np.arange(128)[:, None]
    c_ = np.arange(SB)[None, :]
    mask = (c_ >= r_).astype(np.float32).astype(bf16)
    xt_b = [np.ascontiguousarray(x[b].T.astype(bf16)) for b in range(B)]
    in_maps = []
    for core in range(NCORES):
        b, g = divmod(core, HKV)
        wqkv = np.concatenate([
            Wq[G * DH * g:G * DH * (g + 1)].T,
            Wk[DH * g:DH * (g + 1)].T,
            Wv[DH * g:DH * (g + 1)].T,
        ], axis=1).astype(bf16)
        in_maps.append({
            "xt": xt_b[b],
            "wqkv": np.ascontiguousarray(wqkv),
            "wot": np.ascontiguousarray(
                Wo[:, G * DH * g:G * DH * (g + 1)].T.astype(bf16)),
            "cost": cosT,
            "sints": sints,
            "masks": mask,
        })
    return in_maps


def get_nc():
    if "nc" not in _CACHE:
        _CACHE["nc"] = _build_nc()
    return _CACHE["nc"]


def kernel(x, Wq, Wk, Wv, Wo):
    from concourse.bass_utils import run_bass_kernel_spmd

    nc = get_nc()
    in_maps = build_in_maps(x, Wq, Wk, Wv, Wo)
    res = run_bass_kernel_spmd(nc, in_maps, list(range(NCORES)))
    parts = [res.results[c]["y"].astype(np.float32) for c in range(NCORES)]
    y = np.stack([
        parts[0] + parts[1] + parts[2] + parts[3],
        parts[4] + parts[5] + parts[6] + parts[7],
    ]).astype(np.float32)
    return y


# revision 82
# speedup vs baseline: 1.1016x; 1.0383x over previous
"""GQA (grouped-query attention) Trainium2 kernel, 8-core SPMD.

Sharding: TP=4 over kv-heads x DP=2 over batch  (core = b*4 + g).
Each core computes, for its batch b and kv-head g (q-heads 4g..4g+3):
  QKV projections -> RoPE -> causal softmax(QK^T)V -> partial x@Wo
entirely in transposed layout (feature dim on SBUF partitions), then the
host sums the 4 partial Wo outputs per batch (the TP all-reduce).

Single fused pipeline over the 4 query/sequence blocks of 512:
  proj(sb) -> attention(qb=sb) -> Wo-chunk(qb-1) quarters interleaved
between attention heads so the PE never drains.

Dataflow notes:
 - all matmul operands in bf16 (full PE rate, halves HBM traffic)
 - causal structure: strictly-upper k-blocks skipped; the 4 diagonal
   k-blocks use narrowed moving operands (512/384/256/128 queries) and a
   multiplicative triangular mask after exp
 - softmax runs in S^T[k,q] orientation; denominators: exp tiles are
   accumulated in bf16 into two partial tiles (even k-blocks on the vector
   engine, odd on GPSIMD), reduced across partitions per (head, q-block)
   with gpsimd.partition_all_reduce — the PE does no softmax bookkeeping
 - no max-subtraction: scores are bounded (~+-5) for this problem size
 - V^T -> V[k,dv] reorientation via DMA transpose (16-bit), not the PE
"""

import math
import sys

import numpy as np

if "/opt/trn_rl_repo" not in sys.path:
    sys.path.insert(0, "/opt/trn_rl_repo")

import ml_dtypes

B, S, D = 2, 2048, 2048
HQ, HKV, DH = 16, 4, 128
G = HQ // HKV            # q-heads per kv-head = 4
NCORES = 8
ROPE_THETA = 10000.0
SCALE = 1.0 / math.sqrt(DH)

SB = 512                 # wide column block (moving operand)
NSB = S // SB            # 4
ND = D // 128            # 16 contraction tiles
NKB = S // 128           # 16 key blocks

_CACHE = {}


def _build_nc():
    import concourse.bass as bass
    import concourse.mybir as mybir
    import concourse.tile as tile
    from concourse import bacc

    f32 = mybir.dt.float32
    bf16 = mybir.dt.bfloat16
    AF = mybir.ActivationFunctionType

    nc = bacc.Bacc(
        trn_type="TRN2", target_bir_lowering=False, debug=False,
        num_devices=NCORES,
    )

    xt_d = nc.dram_tensor("xt", [D, S], bf16, kind="ExternalInput").ap()
    wqkv_d = nc.dram_tensor("wqkv", [D, (G + 2) * DH], bf16, kind="ExternalInput").ap()
    wot_d = nc.dram_tensor("wot", [G * DH, D], bf16, kind="ExternalInput").ap()
    cos_d = nc.dram_tensor("cost", [DH, S], bf16, kind="ExternalInput").ap()
    sin_d = nc.dram_tensor("sints", [DH, S], bf16, kind="ExternalInput").ap()
    y_d = nc.dram_tensor("y", [S, D], bf16, kind="ExternalOutput").ap()

    from contextlib import ExitStack

    with tile.TileContext(nc) as tc, ExitStack() as stack, \
            nc.allow_low_precision(reason="bf16 matmul operands"):
        persist = stack.enter_context(tc.tile_pool(name="persist", bufs=1))

        # persistent SBUF tensors (per-block tiles: no reliance on sub-tile
        # dependency tracking)
        xts = [[persist.tile([128, SB], bf16, name=f"x{s}_{i}", tag=f"x{s}_{i}")
                for i in range(ND)] for s in range(NSB)]
        wqkv = [persist.tile([128, (G + 2) * DH], bf16, name=f"w{i}", tag=f"w{i}")
                for i in range(ND)]
        wot = [persist.tile([128, D], bf16, name=f"wo{h}", tag=f"wo{h}")
               for h in range(G)]
        qrt = [[persist.tile([128, SB], bf16, name=f"q{h}_{s}", tag=f"q{h}_{s}")
                for s in range(NSB)] for h in range(G)]
        krt = [persist.tile([128, SB], bf16, name=f"k{s}", tag=f"k{s}")
               for s in range(NSB)]
        vsb = [persist.tile([128, DH], bf16, name=f"v{k}", tag=f"v{k}")
               for k in range(NKB)]
        a_t = [[persist.tile([128, SB], bf16, name=f"a{h}_{s}", tag=f"a{h}_{s}")
                for s in range(NSB)] for h in range(G)]
        cost = [persist.tile([128, SB], bf16, name=f"cost{s}", tag=f"cost{s}")
                for s in range(NSB)]
        sint = [persist.tile([128, SB], bf16, name=f"sint{s}", tag=f"sint{s}")
                for s in range(NSB)]
        mask = persist.tile([128, SB], bf16, name="mask", tag="mask")

        # working rings (SBUF)
        p_pool = stack.enter_context(tc.tile_pool(name="pp", bufs=4))
        dacc_pool = stack.enter_context(tc.tile_pool(name="dap", bufs=2))
        vsbw_pool = stack.enter_context(tc.tile_pool(name="vsw", bufs=2))
        rope_pool = stack.enter_context(tc.tile_pool(name="rope", bufs=2))
        fin_pool = stack.enter_context(tc.tile_pool(name="fin", bufs=2))
        yt_pool = stack.enter_context(tc.tile_pool(name="yt", bufs=2))

        # PSUM: mm(3) shared by proj/Wo + sps(3) + aps(2) = 8 banks
        mm_ps = stack.enter_context(tc.tile_pool(name="mm_ps", bufs=3, space="PSUM"))
        s_ps = stack.enter_context(tc.tile_pool(name="s_ps", bufs=3, space="PSUM"))
        a_ps = stack.enter_context(tc.tile_pool(name="a_ps", bufs=2, space="PSUM"))

        # weights + first x block, interleaved so matmul i can start early;
        # the first stripe ships its V-columns first (first PE matmul)
        nc.sync.dma_start(wqkv[0][:, (G + 1) * DH:], wqkv_d[0:128, (G + 1) * DH:])
        nc.sync.dma_start(xts[0][0][:], xt_d[0:128, 0:SB])
        nc.sync.dma_start(wqkv[0][:, 0:(G + 1) * DH], wqkv_d[0:128, 0:(G + 1) * DH])
        for i in range(1, ND):
            nc.sync.dma_start(wqkv[i][:], wqkv_d[128 * i:128 * (i + 1), :])
            nc.sync.dma_start(xts[0][i][:], xt_d[128 * i:128 * (i + 1), 0:SB])
        # first rope-table chunk right behind (gates the first rope_evict);
        # the rest follows in proj_block(0)
        nc.sync.dma_start(cost[0][:], cos_d[:, 0:SB])
        nc.sync.dma_start(sint[0][:], sin_d[:, 0:SB])
        # causal mask generated on the idle Pool engine at t~0 (keeps
        # 128KB out of the congested startup DMA window): mask[r,c]=1 iff c>=r
        nc.gpsimd.memset(mask[:], 0.0)
        nc.gpsimd.affine_select(
            out=mask[:], in_=mask[:], compare_op=mybir.AluOpType.is_gt,
            fill=1.0, base=0, pattern=[[-1, SB]], channel_multiplier=1)

        def rope_evict(ps, out_tile, sb):
            # Cross-half (rotate-half) reads live on the PSUM operand: the
            # BIR verifier only requires equal base partitions when BOTH
            # tensor_tensor inputs are in SBUF. Muls read PSUM -> DVE; the
            # final add is SBUF-only and partition-aligned -> Pool engine.
            ts_ = rope_pool.tile([128, SB], f32, name="tsin", tag="tsin")
            tcs = rope_pool.tile([128, SB], f32, name="tcos", tag="tcos")
            nc.vector.tensor_mul(ts_[0:64, :], ps[64:128, :], sint[sb][0:64, :])
            nc.vector.tensor_mul(ts_[64:128, :], ps[0:64, :], sint[sb][64:128, :])
            nc.vector.tensor_mul(tcs[:], ps[:], cost[sb][:, :])
            nc.gpsimd.tensor_add(out_tile[:], tcs[:], ts_[:])

        def proj_block(sb):
            c0 = SB * sb
            xt = xts[sb]
            KO, VO = G * DH, (G + 1) * DH  # column offsets of Wk / Wv

            def mm_group(lo, interleaved=()):
                ps = mm_ps.tile([128, SB], f32, name="pp", tag="pp")
                for i in range(ND):
                    nc.tensor.matmul(
                        ps[:], wqkv[i][:, lo:lo + DH], xt[i][:],
                        start=(i == 0), stop=(i == ND - 1),
                        skip_group_check=bool(interleaved))
                return ps

            if sb == 0:
                # x0 tiles arrive at DMA pace: interleave all 6 psum groups
                # by contraction index so the PE rides the DMA wave,
                # borrowing the (not yet used) attention PSUM banks
                pools = [mm_ps, mm_ps, mm_ps, s_ps, s_ps, a_ps]
                tags = ["pp", "pp", "pp", "sps", "sps", "aps"]
                offsets = [VO, KO, 0, DH, 2 * DH, 3 * DH]
                pss = [pool.tile([128, SB], f32, name="pp", tag=t)
                       for pool, t in zip(pools, tags)]
                for i in range(ND):
                    for ps, lo in zip(pss, offsets):
                        nc.tensor.matmul(
                            ps[:], wqkv[i][:, lo:lo + DH], xt[i][:],
                            start=(i == 0), stop=(i == ND - 1),
                            skip_group_check=True)
            def v_evict_transpose(vps):
                # chunked evict + immediate transpose DMA: each vsb[] tile
                # becomes ready as early as possible (the list scheduler
                # slots SP work by readiness, and the PV matmuls need these)
                v_sb = vsbw_pool.tile([128, SB], bf16, name="vsb", tag="vsb")
                for c in range(SB // 128):
                    cc = slice(128 * c, 128 * (c + 1))
                    nc.scalar.copy(v_sb[:, cc], vps[:, cc])
                    nc.sync.dma_start_transpose(vsb[4 * sb + c][:], v_sb[:, cc])

            if sb == 0:
                vps, kps, qps0, qps1, qps2, qps3 = pss
                v_evict_transpose(vps)
                rope_evict(kps, krt[sb], sb)
                rope_evict(qps0, qrt[0][sb], sb)
                rope_evict(qps1, qrt[1][sb], sb)
                rope_evict(qps2, qrt[2][sb], sb)
                rope_evict(qps3, qrt[3][sb], sb)
            else:
                vps = mm_group(VO)
                kps = mm_group(KO)
                v_evict_transpose(vps)
                qps0 = mm_group(0)
                rope_evict(kps, krt[sb], sb)
                qps1 = mm_group(DH)
                rope_evict(qps0, qrt[0][sb], sb)
                qps2 = mm_group(2 * DH)
                rope_evict(qps1, qrt[1][sb], sb)
                qps3 = mm_group(3 * DH)
                rope_evict(qps2, qrt[2][sb], sb)
                rope_evict(qps3, qrt[3][sb], sb)

            # DMA schedule: tables/weights for upcoming phases, next x block
            if sb + 1 < NSB:
                for i in range(ND):
                    nc.sync.dma_start(
                        xts[sb + 1][i][:],
                        xt_d[128 * i:128 * (i + 1), SB * (sb + 1):SB * (sb + 2)])
                    if sb == 0 and i == 7:
                        # rope tables for block 1 slip in mid-bulk: early
                        # enough for proj(1)'s K rope, without starving the
                        # PE of x1 tiles
                        nc.sync.dma_start(cost[1][:], cos_d[:, SB:2 * SB])
                        nc.sync.dma_start(sint[1][:], sin_d[:, SB:2 * SB])
            if sb == 0:
                for s in range(2, NSB):
                    nc.sync.dma_start(cost[s][:], cos_d[:, SB * s:SB * (s + 1)])
                    nc.sync.dma_start(sint[s][:], sin_d[:, SB * s:SB * (s + 1)])
            if sb == 1:
                for h in range(G):
                    nc.sync.dma_start(wot[h][:], wot_d[128 * h:128 * (h + 1), :])

        # deferred finalize steps, drained one per attention block
        fin_steps = []

        def drain_one():
            if fin_steps:
                step = fin_steps.pop(0)
                if step is not None:
                    step()

        def drain_all():
            while fin_steps:
                step = fin_steps.pop(0)
                if step is not None:
                    step()

        def finalize_lazy(h, qb, aps, dacc, daccp):
            """Denominator partition-reduce (Pool) -> reciprocal -> normalize,
            as deferred steps.

            Each step is drained with spacing so cross-engine latency
            (DVE->Pool->DVE) never stalls the PE. partition_all_reduce leaves
            the sum on ALL partitions, so no broadcast step is needed.
            """
            from concourse import bass_isa
            st = {}

            def s0():
                nc.gpsimd.tensor_add(daccp[:], daccp[:], dacc[:])

            def s1():
                dall = fin_pool.tile([128, SB], f32, name="dall", tag="dall")
                nc.gpsimd.partition_all_reduce(
                    dall[:], daccp[:], channels=128,
                    reduce_op=bass_isa.ReduceOp.add)
                st["dall"] = dall

            def s2():
                rbc = fin_pool.tile([128, SB], f32, name="rbc", tag="rbc")
                nc.vector.reciprocal(rbc[:], st["dall"][:])
                st["rbc"] = rbc

            def s3():
                nc.vector.tensor_mul(a_t[h][qb][:], aps[:], st["rbc"][:])

            fin_steps.extend([s0, s1, None, s2, None, s3])

        def attn_head(h, qb, defer_finalize=False):
            """scores -> exp -> (mask) -> dacc accumulate -> PV accumulate.

            PE emission has one-block lookahead: scores(kb+1) before PV(kb).
            """
            nkb = 4 * qb + 4
            aps = a_ps.tile([128, SB], f32, name="aps", tag="aps")
            # two partial denominator accumulators: even k-blocks on DVE,
            # odd on Pool (both SBUF-only, legal for GPSIMD); combined at
            # the end on Pool
            dacc = dacc_pool.tile([128, SB], bf16, name="dacc", tag="dacc")
            daccp = dacc_pool.tile([128, SB], bf16, name="daccp", tag="daccp")
            pend = []  # (kb, p, qoff, w), lookahead-2 queue
            lookahead = 2
            order = list(range(nkb))
            seen = {0: 0, 1: 0}  # per-accumulator emission count
            for kb in order:
                j = kb - 4 * qb
                qoff = 128 * j if j > 0 else 0
                w = SB - qoff
                sps = s_ps.tile([128, SB], f32, name="sps", tag="sps")
                nc.tensor.matmul(
                    sps[:, 0:w], krt[kb // 4][:, 128 * (kb % 4):128 * (kb % 4 + 1)],
                    qrt[h][qb][:, qoff:SB],
                    start=True, stop=True, skip_group_check=True)
                p = p_pool.tile([128, SB], bf16, name="p", tag="p")
                nc.scalar.activation(p[:, 0:w], sps[:, 0:w], AF.Exp, scale=SCALE)
                if j >= 0:
                    nc.vector.tensor_mul(p[:, 0:w], p[:, 0:w], mask[:, 0:w])
                eng, acc = ((nc.vector, dacc) if kb % 2 == 0
                            else (nc.gpsimd, daccp))
                if seen[kb % 2] == 0:
                    if qoff:
                        eng.memset(acc[:, 0:qoff], 0.0)
                    eng.tensor_copy(acc[:, qoff:SB], p[:, 0:w])
                else:
                    eng.tensor_add(acc[:, qoff:SB], acc[:, qoff:SB], p[:, 0:w])
                seen[kb % 2] += 1
                if len(pend) == lookahead:
                    pkb, pp, pqoff, pw = pend.pop(0)
                    nc.tensor.matmul(
                        aps[:, pqoff:SB], vsb[pkb][:], pp[:, 0:pw],
                        start=(pkb == order[0]), stop=False,
                        skip_group_check=True)
                pend.append((kb, p, qoff, w))
                drain_one()
            while pend:
                pkb, pp, pqoff, pw = pend.pop(0)
                nc.tensor.matmul(
                    aps[:, pqoff:SB], vsb[pkb][:], pp[:, 0:pw],
                    start=(pkb == order[0]), stop=(not pend),
                    skip_group_check=True)
            if defer_finalize:
                return aps, dacc, daccp
            finalize_lazy(h, qb, aps, dacc, daccp)
            return None

        def wo_quarter(qb, c, final=False, evict_act=False):
            """y rows [128c'..] for query block qb, quarter c: 4 eb psums.

            Mid-attention quarters evict on DVE (Act runs at exactly PE pace
            there with exp work); endgame quarters evict on Act (idle after
            the last exp, and DVE holds the finalize chains). The final
            quarter DMAs per-eb to shorten the tail.
            """
            yt = yt_pool.tile([128, D], bf16, name="yt", tag="yt")
            sb128 = 4 * qb + c
            for eb in range(NSB):
                yp = mm_ps.tile([128, SB], f32, name="pp", tag="pp")
                for h in range(G):
                    nc.tensor.matmul(
                        yp[:], a_t[h][qb][:, 128 * c:128 * (c + 1)],
                        wot[h][:, SB * eb:SB * (eb + 1)],
                        start=(h == 0), stop=(h == G - 1))
                if final:
                    # per-eb evict+DMA pipeline to shorten the tail
                    nc.scalar.copy(yt[:, SB * eb:SB * (eb + 1)], yp[:])
                    nc.sync.dma_start(
                        y_d[128 * sb128:128 * (sb128 + 1),
                            SB * eb:SB * (eb + 1)],
                        yt[:, SB * eb:SB * (eb + 1)])
                elif evict_act:
                    nc.scalar.copy(yt[:, SB * eb:SB * (eb + 1)], yp[:])
                else:
                    nc.vector.tensor_copy(yt[:, SB * eb:SB * (eb + 1)], yp[:])
            if not final:
                nc.sync.dma_start(
                    y_d[128 * sb128:128 * (sb128 + 1), :], yt[:])

        last = None
        for sb in range(NSB):
            proj_block(sb)
            drain_all()  # a_t[*][sb-1] must be written before wo_quarter reads
            for h in range(G):
                if sb > 0:
                    wo_quarter(sb - 1, h)
                last = attn_head(h, sb,
                                 defer_finalize=(sb == NSB - 1 and h == G - 1))
        drain_all()

        # last head's finalize is latency-critical (it gates the final Wo
        # chunk): run it at 128-column granularity, pipelined against the
        # final Wo quarters
        from concourse import bass_isa
        aps, dacc, daccp = last
        qb = NSB - 1
        dall = fin_pool.tile([128, SB], f32, name="dall", tag="dall")
        for c in range(G):
            cs = slice(128 * c, 128 * (c + 1))
            nc.gpsimd.tensor_add(daccp[:, cs], daccp[:, cs], dacc[:, cs])
            nc.gpsimd.partition_all_reduce(
                dall[:, cs], daccp[:, cs], channels=128,
                reduce_op=bass_isa.ReduceOp.add)
            rbc = fin_pool.tile([128, 128], f32, name="rbcc", tag="rbcc")
            nc.vector.reciprocal(rbc[:], dall[:, cs])
            nc.vector.tensor_mul(a_t[G - 1][qb][:, cs], aps[:, cs], rbc[:])
            if c == 0:
                # wave-open: the h0..h2 accumulation of the first quarter
                # does not depend on the chunk finalize above, so it fills
                # the cross-engine chain latency on the PE
                yt = yt_pool.tile([128, D], bf16, name="yt", tag="yt")
                open_ps = []
                for eb in range(3):
                    yp = mm_ps.tile([128, SB], f32, name="pp", tag="pp")
                    for h in range(G - 1):
                        nc.tensor.matmul(
                            yp[:], a_t[h][qb][:, cs],
                            wot[h][:, SB * eb:SB * (eb + 1)],
                            start=(h == 0), stop=False, skip_group_check=True)
                    open_ps.append(yp)
                for eb in range(3):
                    nc.tensor.matmul(
                        open_ps[eb][:], a_t[G - 1][qb][:, cs],
                        wot[G - 1][:, SB * eb:SB * (eb + 1)],
                        start=False, stop=True, skip_group_check=True)
                    nc.scalar.copy(
                        yt[:, SB * eb:SB * (eb + 1)], open_ps[eb][:])
                yp = mm_ps.tile([128, SB], f32, name="pp", tag="pp")
                for h in range(G):
                    nc.tensor.matmul(
                        yp[:], a_t[h][qb][:, cs], wot[h][:, 3 * SB:4 * SB],
                        start=(h == 0), stop=(h == G - 1))
                nc.scalar.copy(yt[:, 3 * SB:4 * SB], yp[:])
                nc.sync.dma_start(
                    y_d[128 * 4 * qb:128 * (4 * qb + 1), :], yt[:])
            else:
                wo_quarter(qb, c, final=(c == G - 1), evict_act=True)

    nc.compile()
    return nc


def _rope_tables():
    inv = 1.0 / (ROPE_THETA ** (np.arange(0, DH, 2, dtype=np.float64) / DH))
    pos = np.arange(S, dtype=np.float64)
    theta = np.concatenate([np.outer(pos, inv)] * 2, axis=1)  # [S, DH]
    cosT = np.cos(theta).T.astype(np.float32)                 # [DH, S]
    sinT = np.sin(theta).T.astype(np.float32)
    sints = np.concatenate([-sinT[:64], sinT[64:]], axis=0)
    bf16 = ml_dtypes.bfloat16
    return (np.ascontiguousarray(cosT.astype(bf16)),
            np.ascontiguousarray(sints.astype(bf16)))


def build_in_maps(x, Wq, Wk, Wv, Wo):
    bf16 = ml_dtypes.bfloat16
    x = np.asarray(x, np.float32)
    Wq = np.asarray(Wq, np.float32)
    Wk = np.asarray(Wk, np.float32)
    Wv = np.asarray(Wv, np.float32)
    Wo = np.asarray(Wo, np.float32)
    cosT, sints = _rope_tables()
    xt_b = [np.ascontiguousarray(x[b].T.astype(bf16)) for b in range(B)]
    in_maps = []
    for core in range(NCORES):
        b, g = divmod(core, HKV)
        wqkv = np.concatenate([
            Wq[G * DH * g:G * DH * (g + 1)].T,
            Wk[DH * g:DH * (g + 1)].T,
            Wv[DH * g:DH * (g + 1)].T,
        ], axis=1).astype(bf16)
        in_maps.append({
            "xt": xt_b[b],
            "wqkv": np.ascontiguousarray(wqkv),
            "wot": np.ascontiguousarray(
                Wo[:, G * DH * g:G * DH * (g + 1)].T.astype(bf16)),
            "cost": cosT,
            "sints": sints,
        })
    return in_maps


def get_nc():
    if "nc" not in _CACHE:
        _CACHE["nc"] = _build_nc()
    return _CACHE["nc"]


def kernel(x, Wq, Wk, Wv, Wo):
    from concourse.bass_utils import run_bass_kernel_spmd

    nc = get_nc()
    in_maps = build_in_maps(x, Wq, Wk, Wv, Wo)
    res = run_bass_kernel_spmd(nc, in_maps, list(range(NCORES)))
    parts = [res.results[c]["y"].astype(np.float32) for c in range(NCORES)]
    y = np.stack([
        parts[0] + parts[1] + parts[2] + parts[3],
        parts[4] + parts[5] + parts[6] + parts[7],
    ]).astype(np.float32)
    return y
